# revision 44
# baseline (speedup 1.0000x reference)
"""Trainium2 Bass kernel for nn_ContextAttentionAdapterWrapper — v8.

Track-sharded (8 tracks/core) attention with the j axis on PSUM
partitions: per (head, j-chunk) one sim matmul produces
simT [j_rows<=128, 896], exp runs on Scalar with the kv-LN scale as the
ACT per-partition scale operand, and the softmax den/num reductions are
PE matmuls with per-track block weights (den: 0/1 masks, num: masked
vw = v . (Wo@Wp) columns) accumulated across chunks into a shared
[128, 2x512] PSUM tile.  The output head-sum, the embeddings@Wp row and
softplus close it out; the null token never enters the j axis (one tiny
matmul + exp, folded per head).

Cross-core: q is hybrid-replicated.  Head-pair 0's q projection (and the
full-n stats) are computed on every core — that work fills the latency
of an 8-rank AllGather that carries the other three head-pairs' q
(each core projects only its 112-column n-slice).  The main loop runs
four 2-head batches; batch 0 depends only on local q, batches 1-3 on
the gather.  Loads: q-side chain on the sync HWDGE queue (so the
gather's bounce DMA issues early), ctx-side + in-loop small DMAs on the
scalar queue (so they never sit behind the cc-blocked unpack).
"""

import os
import sys

for _p in ("/opt/trn_rl_repo", "/root/.axon_site/_ro/trn_rl_repo"):
    if os.path.isdir(_p) and _p not in sys.path:
        sys.path.append(_p)

import numpy as np

import concourse.bass as bass
import concourse.tile as tile
from concourse import bacc, mybir
from concourse.bass_utils import run_bass_kernel_spmd
from concourse.tile_rust import add_dep_helper

F32 = mybir.dt.float32
F16 = mybir.dt.float16
AF = mybir.ActivationFunctionType
AX = mybir.AxisListType
OP = mybir.AluOpType

N_CORES = 8
B, N, DH = 1, 896, 3072
C, J, DC = 64, 127, 1024
H, D = 8, 64
INNER = H * D
EPS = 1e-5
SCALE = D ** -0.5
TPC = C // N_CORES
KQ = DH // 128
KKV = DC // 128
NIQ = INNER // 128
NKH = NIQ
NH = N // 2
NSL = N // N_CORES            # 112 q columns per core
QSH = 3 * 128 * NSL           # f16 elements of the q shard (its 1..3)

_BUILD_CACHE = {}
LAST_RESULTS = None


def _dep(after, *befores):
    for b in befores:
        add_dep_helper(after.ins, b.ins, sync=True, reason="dram bounce order")


def _build(jeff):
    JT = TPC * jeff
    G = -(-JT // 128)
    cws = [min(128, JT - 128 * g) for g in range(G)]
    nc = bacc.Bacc("TRN2", target_bir_lowering=False, debug=False,
                   enable_asserts=False, num_devices=N_CORES)

    embT = nc.dram_tensor("embT", [128, KQ, NSL], F16, kind="ExternalInput").ap()
    embF = nc.dram_tensor("embF", [128, KQ, N], F16, kind="ExternalInput").ap()
    wq = nc.dram_tensor("wq", [NIQ, 128, KQ, 128], F16, kind="ExternalInput").ap()
    augq = nc.dram_tensor("augq", [2, NIQ, 128], F16, kind="ExternalInput").ap()
    onesw = nc.dram_tensor("onesw", [128, KQ, 2], F16, kind="ExternalInput").ap()
    wkv = nc.dram_tensor("wkv", [128, NKH, KKV, 128], F16, kind="ExternalInput").ap()
    augkv = nc.dram_tensor("augkv", [2, NKH, 128], F16, kind="ExternalInput").ap()
    wv = nc.dram_tensor("wv", [128, KKV, H], F16, kind="ExternalInput").ap()
    augv = nc.dram_tensor("augv", [2, H], F16, kind="ExternalInput").ap()
    ctxT = nc.dram_tensor("ctxT", [128, KKV, JT], F16, kind="ExternalInput").ap()
    nullk = nc.dram_tensor("nullk", [128, NIQ, 2], F16, kind="ExternalInput").ap()
    nullw = nc.dram_tensor("nullw", [H, H, 16], F16, kind="ExternalInput").ap()
    wredb = nc.dram_tensor("wredb", [128, G, H, 16], F16, kind="ExternalInput").ap()
    hw = nc.dram_tensor("hw", [INNER // H + 1, H], F32, kind="ExternalInput").ap()
    consts = nc.dram_tensor("consts", [1, 4], F32, kind="ExternalInput").ap()
    out_d = nc.dram_tensor("out", [TPC, N], F32, kind="ExternalOutput").ap()

    from contextlib import ExitStack
    with tile.TileContext(nc) as tc, ExitStack() as ctx:
        const = ctx.enter_context(tc.tile_pool(name="const", bufs=1))
        dram = ctx.enter_context(tc.tile_pool(name="dram", bufs=1, space="DRAM"))
        sq_pool = ctx.enter_context(tc.tile_pool(name="sqp", bufs=2))
        expp = ctx.enter_context(tc.tile_pool(name="expp", bufs=4))
        psA = ctx.enter_context(tc.tile_pool(name="psA", bufs=3, space="PSUM"))
        ps_nd = ctx.enter_context(tc.tile_pool(name="ps_nd", bufs=1, space="PSUM"))

        # ---- loads: q-side chain on sync (w_q early), ctx-side on scalar ----
        consts_sb = const.tile([128, 4], F32)
        nc.sync.dma_start(out=consts_sb[:],
                          in_=bass.AP(tensor=consts.tensor, offset=consts.offset,
                                      ap=[[0, 128], [1, 4]]))
        emb_sb = const.tile([128, KQ, NSL], F16)
        nc.sync.dma_start(out=emb_sb[:], in_=embT[:])
        onesw_sb = const.tile([128, KQ, 2], F16)
        nc.sync.dma_start(out=onesw_sb[:], in_=onesw[:])
        augq_sb = const.tile([1, 2, NIQ, 128], F16)
        nc.sync.dma_start(out=augq_sb[:], in_=augq.rearrange("r t i -> (r t i)"))
        wq_sb = const.tile([128, NIQ, KQ, 128], F16)
        for it in (1, 2, 3):
            nc.sync.dma_start(out=wq_sb[:, it], in_=wq[it])
        ctx_sb = const.tile([128, KKV, 2, JT], F16)
        nc.scalar.dma_start(out=ctx_sb[:, :, 0, :], in_=ctxT[:])
        wkv_sb = const.tile([128, NKH, KKV, 128], F16)
        nc.scalar.dma_start(out=wkv_sb[:], in_=wkv[:])
        augkv_sb = const.tile([1, 2, NKH, 128], F16)
        nc.scalar.dma_start(out=augkv_sb[:], in_=augkv.rearrange("r t i -> (r t i)"))
        wv_sb = const.tile([128, KKV, H], F16)
        nc.scalar.dma_start(out=wv_sb[:], in_=wv[:])
        augv_sb = const.tile([1, 2, H], F16)
        nc.scalar.dma_start(out=augv_sb[:], in_=augv.rearrange("r h -> (r h)"))
        wredb_sb = const.tile([128, G, H, 16], F16)
        nc.scalar.dma_start(out=wredb_sb[:], in_=wredb[:])
        nullw_sb = const.tile([H, H, 16], F16)
        nc.scalar.dma_start(out=nullw_sb[:], in_=nullw[:])
        hw_sb = const.tile([D + 1, H], F32)
        nc.scalar.dma_start(out=hw_sb[:], in_=hw[:])
        nullk_sb = const.tile([128, NIQ, 2], F16)
        nc.scalar.dma_start(out=nullk_sb[:], in_=nullk[:])
        embf_sb = const.tile([128, KQ, N], F16)
        KH2 = KQ // 2

        def emit_late_loads():
            nc.sync.dma_start(out=wq_sb[:, 0], in_=wq[0])
            nc.sync.dma_start(out=embf_sb[:, 0:KH2, :],
                              in_=embF[:, 0:KH2, :])
            nc.sync.dma_start(out=embf_sb[:, KH2:KQ, :],
                              in_=embF[:, KH2:KQ, :])
        onedc = const.tile([128, 1], F16)
        nc.vector.memset(onedc[:], 1.0 / DC)
        onedh = const.tile([128, 1], F16)
        nc.vector.memset(onedh[:], 1.0 / DH)
        warm2 = const.tile([1, 4], F32)
        nc.scalar.activation(out=warm2[:], in_=consts_sb[0:1, 0:4], func=AF.Sqrt)
        nc.vector.tensor_mul(ctx_sb[:, :, 1, :], ctx_sb[:, :, 0, :],
                             ctx_sb[:, :, 0, :])

        # ---------------- q stats (local 112-column slice) ----------------
        rows = const.tile([1, 4, NSL], F32)
        mu_r = rows[0:1, 0, :]
        var_r = rows[0:1, 1, :]
        sig_r = rows[0:1, 2, :]
        s_r = rows[0:1, 3, :]
        sum_ps = psA.tile([128, 2, 512], F32, name="ps")[0:2, 0, 0:NSL]
        sq_ps = psA.tile([128, 2, 512], F32, name="ps")[0:1, 0, 0:NSL]
        for g in range(2):
            sq = sq_pool.tile([128, KH2, 256], F16, name="sq")[:, :, :NSL]
            nc.vector.tensor_mul(sq[:], emb_sb[:, g * KH2:(g + 1) * KH2, :],
                                 emb_sb[:, g * KH2:(g + 1) * KH2, :])
            for kk in range(KH2):
                k = g * KH2 + kk
                nc.tensor.matmul(sum_ps, onesw_sb[:, k, :], emb_sb[:, k, :],
                                 start=(k == 0), stop=(k == KQ - 1))
                nc.tensor.matmul(sq_ps, onedh[:], sq[:, kk, :],
                                 start=(k == 0), stop=(k == KQ - 1))
        nc.vector.tensor_copy(mu_r, sum_ps[0:1, :])
        nc.vector.tensor_copy(var_r, sq_ps)
        nc.vector.tensor_mul(sig_r, mu_r, mu_r)
        nc.vector.tensor_sub(var_r, var_r, sig_r)
        nc.scalar.activation(out=sig_r, in_=var_r, func=AF.Sqrt,
                             bias=consts_sb[0:1, 0:1])
        nc.vector.reciprocal_approx_fast(s_r, sig_r)
        nc.vector.tensor_scalar_mul(s_r, s_r, SCALE)
        mu16 = const.tile([1, NSL], F16)
        nc.vector.tensor_copy(mu16[:], mu_r)
        sig16 = const.tile([1, NSL], F16)
        nc.vector.tensor_copy(sig16[:], sig_r)
        s_bc = const.tile([128, NSL], F32)
        nc.gpsimd.partition_broadcast(s_bc[:], s_r, channels=128)

        # ---------------- local q slice projection (its 1..3) ----------------
        qsl = const.tile([128, 3, NSL], F16)
        for it in range(1, NIQ):
            q_ps = psA.tile([128, 2, 512], F32, name="ps")[:, 0, 0:NSL]
            for k in range(KQ):
                nc.tensor.matmul(q_ps, wq_sb[:, it, k, :], emb_sb[:, k, :],
                                 start=(k == 0), stop=False)
            nc.tensor.matmul(q_ps, augq_sb[0:1, 0, it, :], mu16[:],
                             start=False, stop=False)
            nc.tensor.matmul(q_ps, augq_sb[0:1, 1, it, :], sig16[:],
                             start=False, stop=True)
            nc.vector.tensor_mul(qsl[:, it - 1, :], q_ps, s_bc[:])

        # ---------------- all-gather q (its 1..3) across the 8 cores --------
        agin = dram.tile([QSH], F16)
        agout = dram.tile([N_CORES * QSH], F16, addr_space="Shared")
        w_q = nc.sync.dma_start(
            out=bass.AP(tensor=agin.tensor, offset=agin.offset,
                        ap=[[NSL, 128], [128 * NSL, 3], [1, NSL]]),
            in_=qsl[:])
        cc = nc.gpsimd.collective_compute(
            "AllGather", mybir.AluOpType.bypass,
            replica_groups=[list(range(N_CORES))],
            ins=[agin.opt()], outs=[agout.opt()])
        _dep(cc, w_q)
        emit_late_loads()

        # ---------------- ctx stats (overlaps the all-gather) ----------------
        crows = const.tile([1, 3, JT], F32)
        cmu_r = crows[0:1, 0, :]
        cvar_r = crows[0:1, 1, :]
        csig_r = crows[0:1, 2, :]
        csc_r = cvar_r
        cch = [(o, min(256, JT - o)) for o in range(0, JT, 256)]
        for (o, w) in cch:
            cst = psA.tile([128, 2, 512], F32, name="ps")[0:1, 0, :]
            for k in range(KKV):
                nc.tensor.matmul(cst[:, 0:2 * w].rearrange("p (r x) -> p r x", r=2),
                                 onedc[:], ctx_sb[:, k, :, o:o + w],
                                 start=(k == 0), stop=(k == KKV - 1))
            nc.vector.tensor_copy(cmu_r[:, o:o + w], cst[:, 0:w])
            nc.vector.tensor_copy(cvar_r[:, o:o + w], cst[:, w:2 * w])
        nc.vector.tensor_mul(csig_r, cmu_r, cmu_r)
        nc.vector.tensor_sub(cvar_r, cvar_r, csig_r)
        nc.scalar.activation(out=csig_r, in_=cvar_r, func=AF.Sqrt,
                             bias=consts_sb[0:1, 0:1])
        nc.vector.reciprocal_approx_fast(csc_r, csig_r)
        cmu16 = const.tile([1, JT], F16)
        nc.vector.tensor_copy(cmu16[:], cmu_r)
        csig16 = const.tile([1, JT], F16)
        nc.vector.tensor_copy(csig16[:], csig_r)
        csc_d = dram.tile([128 * G], F32)
        w_csc = nc.sync.dma_start(out=csc_d[0:JT], in_=csc_r)
        csc_col = const.tile([128, G], F32)
        w_cscc = nc.sync.dma_start(
            out=csc_col[:],
            in_=bass.AP(tensor=csc_d.tensor, offset=csc_d.offset,
                        ap=[[1, 128], [128, G]]))
        _dep(w_cscc, w_csc)

        # ---------------- kv projection (k half, unscaled) ----------------
        kvT = const.tile([128, NKH, JT], F16)
        kch = [(o, min(512, JT - o)) for o in range(0, JT, 512)]
        for it in range(NKH):
            for (fo, fw) in kch:
                kv_ps = psA.tile([128, 2, 512], F32, name="ps")[:, 0, 0:fw]
                for k in range(KKV):
                    nc.tensor.matmul(kv_ps, wkv_sb[:, it, k, :],
                                     ctx_sb[:, k, 0, fo:fo + fw],
                                     start=(k == 0), stop=False)
                nc.tensor.matmul(kv_ps, augkv_sb[0:1, 0, it, :],
                                 cmu16[:, fo:fo + fw], start=False, stop=False)
                nc.tensor.matmul(kv_ps, augkv_sb[0:1, 1, it, :],
                                 csig16[:, fo:fo + fw], start=False, stop=True)
                nc.vector.tensor_copy(kvT[:, it, fo:fo + fw], kv_ps)

        # ---------------- vwT columns + Wred num cols ----------------
        vwT = const.tile([128, G, H], F32)
        for g in range(G):
            cw = cws[g]
            gsl = slice(128 * g, 128 * g + cw)
            vw_ps = psA.tile([128, 2, 512], F32, name="ps")[0:cw, 0, 0:H]
            for k in range(KKV):
                nc.tensor.matmul(vw_ps, ctx_sb[:, k, 0, gsl], wv_sb[:, k, :],
                                 start=(k == 0), stop=False)
            nc.tensor.matmul(vw_ps, cmu16[:, gsl], augv_sb[0:1, 0, :],
                             start=False, stop=False)
            nc.tensor.matmul(vw_ps, csig16[:, gsl], augv_sb[0:1, 1, :],
                             start=False, stop=True)
            nc.vector.tensor_scalar_mul(vwT[0:cw, g, :], vw_ps,
                                        csc_col[0:cw, g:g + 1])
        for g in range(G):
            cw = cws[g]
            for h in range(H):
                nc.vector.tensor_scalar_mul(
                    wredb_sb[0:cw, g, h, 8:16], wredb_sb[0:cw, g, h, 0:8],
                    vwT[0:cw, g, h:h + 1])

        # ---------------- full q stats + head-pair-0 projection ----------
        qT_sb = const.tile([128, NIQ, N], F16)
        prod = const.tile([D + 1, 2, NH], F32)
        qch = [(o, min(256, N - o)) for o in range(0, N, 256)]
        rowsF = const.tile([1, 4, N], F32)
        muF_r = rowsF[0:1, 0, :]
        varF_r = rowsF[0:1, 1, :]
        sigF_r = rowsF[0:1, 2, :]
        sF_r = rowsF[0:1, 3, :]
        ep2F = const.tile([2, N], F32)
        for (o, w) in qch:
            sl = slice(o, o + w)
            sumF_ps = psA.tile([128, 2, 512], F32, name="ps")[0:2, 0, 0:w]
            sqF_ps = psA.tile([128, 2, 512], F32, name="ps")[0:1, 0, 0:w]
            for g in range(2):
                sqf = sq_pool.tile([128, KH2, 256], F16, name="sq")[:, :, :w]
                nc.vector.tensor_mul(sqf[:],
                                     embf_sb[:, g * KH2:(g + 1) * KH2, sl],
                                     embf_sb[:, g * KH2:(g + 1) * KH2, sl])
                for kk in range(KH2):
                    k = g * KH2 + kk
                    nc.tensor.matmul(sumF_ps, onesw_sb[:, k, :],
                                     embf_sb[:, k, sl],
                                     start=(k == 0), stop=(k == KQ - 1))
                    nc.tensor.matmul(sqF_ps, onedh[:], sqf[:, kk, :],
                                     start=(k == 0), stop=(k == KQ - 1))
            nc.vector.tensor_copy(muF_r[:, sl], sumF_ps[0:1, :])
            nc.vector.tensor_copy(varF_r[:, sl], sqF_ps)
            nc.vector.tensor_copy(ep2F[0:2, sl], sumF_ps[0:2, :])
        nc.vector.tensor_scalar_add(ep2F[0:2, :], ep2F[0:2, :],
                                    consts_sb[0:2, 1:2])
        nc.vector.tensor_mul(sigF_r, muF_r, muF_r)
        nc.vector.tensor_sub(varF_r, varF_r, sigF_r)
        nc.scalar.activation(out=sigF_r, in_=varF_r, func=AF.Sqrt,
                             bias=consts_sb[0:1, 0:1])
        nc.vector.reciprocal_approx_fast(sF_r, sigF_r)
        nc.vector.tensor_scalar_mul(sF_r, sF_r, SCALE)
        warm = const.tile([1, 4], F32)
        nc.scalar.activation(out=warm[:], in_=consts_sb[0:1, 0:4], func=AF.Exp)
        mu16F = const.tile([1, N], F16)
        nc.vector.tensor_copy(mu16F[:], muF_r)
        sig16F = const.tile([1, N], F16)
        nc.vector.tensor_copy(sig16F[:], sigF_r)
        s_bcF = const.tile([128, N], F32)
        nc.gpsimd.partition_broadcast(s_bcF[:], sF_r, channels=128)
        for (o, w) in ((0, NH), (NH, NH)):
            sl = slice(o, o + w)
            q_ps = psA.tile([128, 2, 512], F32, name="ps")[:, 0, 0:w]
            for k in range(KQ):
                nc.tensor.matmul(q_ps, wq_sb[:, 0, k, :], embf_sb[:, k, sl],
                                 start=(k == 0), stop=False)
            nc.tensor.matmul(q_ps, augq_sb[0:1, 0, 0, :], mu16F[:, sl],
                             start=False, stop=False)
            nc.tensor.matmul(q_ps, augq_sb[0:1, 1, 0, :], sig16F[:, sl],
                             start=False, stop=True)
            nc.vector.tensor_mul(qT_sb[:, 0, sl], q_ps, s_bcF[:, sl])
        ep_row = prod[D:D + 1, :, :].rearrange("p a b -> p (a b)")
        nc.sync.dma_start(out=ep_row[:], in_=ep2F[1:2, :])

        # ---------------- unpack gathered q (its 1..3) ----------------
        for m in range(N_CORES):
            eng = nc.sync if m % 2 == 0 else nc.scalar
            r_q = eng.dma_start(
                out=qT_sb[:, 1:4, m * NSL:(m + 1) * NSL],
                in_=bass.AP(tensor=agout.tensor,
                            offset=agout.offset + m * QSH,
                            ap=[[NSL, 128], [128 * NSL, 3], [1, NSL]]))
            _dep(r_q, cc)

        # ---------------- attention main loop ----------------
        nullexp = const.tile([H, 2, NH], F16)
        nullexp_st = const.tile([66, 2, NH], F16)
        ndc = const.tile([64, 2, 2, NH], F32)

        def emit_nullsim0():
            nps = psA.tile([128, 2, 512], F32, name="ps")
            for hf in range(2):
                nc.tensor.matmul(nps[0:2, hf, 0:NH],
                                 nullk_sb[:, 0, :],
                                 qT_sb[:, 0, hf * NH:(hf + 1) * NH],
                                 start=True, stop=True)
            nc.scalar.activation(out=nullexp_st[0:2, :, :],
                                 in_=nps[0:2, :, 0:NH], func=AF.Exp)
            nc.scalar.dma_start(out=nullexp[0:2, :, :],
                                in_=nullexp_st[0:2, :, :])

        def emit_nullsim123():
            nps = psA.tile([128, 2, 512], F32, name="ps")
            for idx, it in enumerate((1, 2, 3)):
                ro = 32 * idx
                for hf in range(2):
                    nc.tensor.matmul(nps[ro:ro + 2, hf, 0:NH],
                                     nullk_sb[:, it, :],
                                     qT_sb[:, it, hf * NH:(hf + 1) * NH],
                                     start=True, stop=True,
                                     tile_position=(0, ro))
            nc.scalar.activation(out=nullexp_st[0:66, :, :],
                                 in_=nps[0:66, :, 0:NH], func=AF.Exp)
            for idx, it in enumerate((1, 2, 3)):
                nc.sync.dma_start(
                    out=nullexp[2 * it:2 * it + 2, :, :],
                    in_=nullexp_st[32 * idx:32 * idx + 2, :, :])

        def emit_head(h, nd):
            it, po = h // 2, 64 * (h % 2)
            q = h % 2
            exps = []
            sims = []

            def emit_sim(g):
                cw = cws[g]
                gsl = slice(128 * g, 128 * g + cw)
                sim_ps = psA.tile([128, 2, 512], F32, name="ps")
                for hf in range(2):
                    nc.tensor.matmul(sim_ps[0:cw, hf, 0:NH],
                                     kvT[po:po + 64, it, gsl],
                                     qT_sb[po:po + 64, it, hf * NH:(hf + 1) * NH],
                                     start=True, stop=True)
                sims.append(sim_ps)

            def emit_exp(g):
                cw = cws[g]
                ex = expp.tile([128, 2, NH], F16, name="exp_sb")
                nc.scalar.activation(out=ex[0:cw, :, :],
                                     in_=sims[g][0:cw, :, 0:NH], func=AF.Exp,
                                     scale=csc_col[0:cw, g:g + 1])
                exps.append(ex)

            def emit_nd(g):
                cw = cws[g]
                for hf in range(2):
                    nc.tensor.matmul(nd[32 * q:32 * q + 16, hf, 0:NH],
                                     wredb_sb[0:cw, g, h, :],
                                     exps[g][0:cw, hf, :],
                                     start=(g == 0), stop=False,
                                     tile_position=(0, 32 * q))

            emit_sim(0)
            emit_sim(1)
            for g in range(G):
                emit_exp(g)
                if g + 2 < G:
                    emit_sim(g + 2)
                emit_nd(g)
            for hf in range(2):
                nc.tensor.matmul(nd[32 * q:32 * q + 16, hf, 0:NH],
                                 nullw_sb[:, h, :], nullexp[:, hf, :],
                                 start=False, stop=True,
                                 tile_position=(0, 32 * q))

        den_sb = const.tile([D, 2, NH], F32)
        num_sb = const.tile([D, 2, NH], F32)
        rden = const.tile([D, 2, NH], F32)
        nc.vector.memset(nullexp[:], 0.0)
        for b in range(NIQ):
            if b == 0:
                emit_nullsim0()
            elif b == 1:
                emit_nullsim123()
            nd = ps_nd.tile([128, 2, 512], F32, name="nd_ps")
            emit_head(2 * b, nd)
            emit_head(2 * b + 1, nd)
            pb = b % 2
            nc.vector.tensor_copy(ndc[:, pb], nd[0:64, :, 0:NH])
            for hq in range(2):
                h = 2 * b + hq
                nc.sync.dma_start(out=den_sb[8 * h:8 * h + 8, :, :],
                                  in_=ndc[32 * hq:32 * hq + 8, pb, :, :])
                nc.sync.dma_start(out=num_sb[8 * h:8 * h + 8, :, :],
                                  in_=ndc[32 * hq + 8:32 * hq + 16, pb, :, :])

        # ---------------- epilogue ----------------
        nc.vector.reciprocal_approx_fast(
            rden[:].rearrange("p a b -> p (a b)"),
            den_sb[:].rearrange("p a b -> p (a b)"))
        nc.vector.tensor_mul(prod[0:D], num_sb[:], rden[:])
        sp = const.tile([TPC, 2, NH], F32)
        spl = const.tile([TPC, 2, NH], F32)
        hs_ps = psA.tile([128, 2, 512], F32, name="ps")
        for hf in range(2):
            nc.tensor.matmul(hs_ps[0:TPC, hf, 0:NH], hw_sb[:], prod[:, hf, :],
                             start=True, stop=True)
        nc.scalar.activation(out=sp[:], in_=hs_ps[0:TPC, :, 0:NH], func=AF.Exp)
        nc.scalar.activation(out=spl[:], in_=sp[:], func=AF.Ln, bias=1.0)
        nc.sync.dma_start(out=out_d[:],
                          in_=spl[:].rearrange("p a b -> p (a b)"))

    nc.compile()
    return nc


def _prep(inputs):
    f16 = np.float16
    emb = np.asarray(inputs["embeddings"], np.float32)[0]
    ctxf = np.asarray(inputs["context"], np.float32)
    km = np.asarray(inputs["context_mask"])[0].astype(bool)
    Wq = np.asarray(inputs["Wq"], np.float32)
    Wkv = np.asarray(inputs["Wkv"], np.float32)
    Wo = np.asarray(inputs["Wo"], np.float32)
    Wp = np.asarray(inputs["Wp"], np.float32)
    qg = np.asarray(inputs["q_gamma"], np.float32)
    qb = np.asarray(inputs["q_beta"], np.float32)
    kg = np.asarray(inputs["kv_gamma"], np.float32)
    kb = np.asarray(inputs["kv_beta"], np.float32)
    nk = np.asarray(inputs["null_k"], np.float32)
    nv = np.asarray(inputs["null_v"], np.float32)
    bo = np.asarray(inputs["bo"], np.float32)
    bp = np.asarray(inputs["bp"], np.float32)

    jeff = int(km.sum())
    assert jeff >= 1
    JT = TPC * jeff
    G = -(-JT // 128)

    wop = (Wo @ Wp)[:, 0]
    c0 = np.float32(bo @ Wp[:, 0] + bp[0])
    Wq_f = qg[:, None] * Wq
    qbias = qb @ Wq
    qcol = Wq_f.sum(0)
    Wkv_f = kg[:, None] * Wkv
    kvbias = kb @ Wkv
    Wk_f = Wkv_f[:, :INNER]
    kcol = Wk_f.sum(0)
    kbias = kvbias[:INNER]
    Wv_f = Wkv_f[:, INNER:]
    wv = np.stack([Wv_f[:, h * D:(h + 1) * D] @ wop[h * D:(h + 1) * D]
                   for h in range(H)], axis=1)
    vbias = np.array([kvbias[INNER + h * D:INNER + (h + 1) * D]
                      @ wop[h * D:(h + 1) * D] for h in range(H)], np.float32)
    vcol = wv.sum(0)
    cnv = np.array([nv[h * D:(h + 1) * D] @ wop[h * D:(h + 1) * D]
                    for h in range(H)], np.float32)

    wq_t = np.ascontiguousarray(
        Wq_f.reshape(KQ, 128, NIQ, 128).transpose(2, 1, 0, 3)).astype(f16)
    augq = np.ascontiguousarray(np.stack(
        [-qcol, qbias]).reshape(2, NIQ, 128)).astype(f16)
    onesw = np.empty((128, KQ, 2), np.float32)
    onesw[:, :, 0] = 1.0 / DH
    onesw[:, :, 1] = Wp[:, 0].reshape(KQ, 128).T
    wkv_t = np.ascontiguousarray(
        Wk_f.reshape(KKV, 128, NKH, 128).transpose(1, 2, 0, 3)).astype(f16)
    augkv = np.ascontiguousarray(np.stack(
        [-kcol, kbias]).reshape(2, NKH, 128)).astype(f16)
    wv_t = np.ascontiguousarray(
        wv.reshape(KKV, 128, H).transpose(1, 0, 2)).astype(f16)
    augv = np.stack([-vcol, vbias]).astype(f16)
    # nullk as [128, NIQ, 2]: col (it, sub) holds head (2it+sub)'s null_k in
    # rows [64*sub, 64*sub+64), zeros elsewhere (masks the other head's d)
    nullk_t = np.zeros((128, NIQ, 2), np.float32)
    for it in range(NIQ):
        for sub in range(2):
            nullk_t[64 * sub:64 * sub + 64, it, sub] = nk[
                (2 * it + sub) * D:(2 * it + sub + 1) * D]
    nullk_t = nullk_t.astype(f16)
    # null fold weights: nullw[h', h, :]: den cols = 1, num cols = cnv_h at h'==h
    nullw = np.zeros((H, H, 16), np.float32)
    for h in range(H):
        nullw[h, h, 0:8] = 1.0
        nullw[h, h, 8:16] = cnv[h]
    # den track masks: wredb[r, g, :, t] = 1 iff global row in track t
    wredb = np.zeros((128, G, H, 16), np.float32)
    for g in range(G):
        for r in range(min(128, JT - 128 * g)):
            t = (128 * g + r) // jeff
            wredb[r, g, :, t] = 1.0
    # head-sum weights: rows 8h+t -> col t; row 64 (ep) -> all 1
    hwm = np.zeros((D + 1, H), np.float32)
    for h in range(H):
        for t in range(H):
            hwm[8 * h + t, t] = 1.0
    hwm[D, :] = 1.0
    consts = np.array([[EPS, c0, 0.0, 0.0]], np.float32)

    embT = np.ascontiguousarray(
        emb.reshape(N, KQ, 128).transpose(2, 1, 0)).astype(f16)  # [128, KQ, N]
    ctx_kept = ctxf[:, km, :]
    shared = {
        "embF": embT,
        "wq": wq_t, "augq": augq, "onesw": onesw.astype(f16),
        "wkv": wkv_t, "augkv": augkv, "wv": wv_t, "augv": augv,
        "nullk": nullk_t, "nullw": nullw.astype(f16),
        "hw": hwm, "consts": consts,
    }
    in_maps = []
    for m in range(N_CORES):
        sh = ctx_kept[m * TPC:(m + 1) * TPC]
        ctxT = np.ascontiguousarray(
            sh.reshape(JT, KKV, 128).transpose(2, 1, 0)).astype(f16)
        in_maps.append(dict(
            shared, ctxT=ctxT, wredb=wredb.astype(f16),
            embT=np.ascontiguousarray(embT[:, :, m * NSL:(m + 1) * NSL])))
    return jeff, in_maps


def kernel(**inputs) -> np.ndarray:
    global LAST_RESULTS
    jeff, in_maps = _prep(inputs)
    if jeff not in _BUILD_CACHE:
        _BUILD_CACHE[jeff] = _build(jeff)
    nc = _BUILD_CACHE[jeff]
    trace = os.environ.get("BASS_KERNEL_TRACE", "") == "1"
    res = run_bass_kernel_spmd(nc, in_maps, core_ids=list(range(N_CORES)),
                               trace=trace)
    LAST_RESULTS = res
    out = np.concatenate([res.results[m]["out"] for m in range(N_CORES)],
                         axis=0)
    return out.T[None].astype(np.float32)
